# revision 2
# baseline (speedup 1.0000x reference)
"""HAN forward pass on 8 TRN2 NeuronCores — v2 (latency-optimized).

Data-parallel over batch (8 docs/core), no collectives. vs v1:
* r/z sigmoids split; per-direction gate chains interleaved so the two
  independent GRU recurrences pipeline across PE/Act/DVE.
* Attention u-projection emitted feature-major (lhsT=Wa^T blocks,
  rhs=hT) one step behind the recurrence; the score reduction is 4
  tiny PE matmuls into PSUM instead of a 512-wide DVE op.
* Word-attention weighted sum via diag(a_t) matmuls accumulating in
  PSUM (produces sent feature-major for the sentence stage directly).
* Sentence stage fully feature-major: batch-8 matmuls run at N=8
  instead of N=512.
"""

import numpy as np
import ml_dtypes

import concourse.bass as bass
import concourse.mybir as mybir
import concourse.tile as tile
from concourse import bacc, bass_utils
from concourse.masks import make_identity

BF = mybir.dt.bfloat16
F32 = mybir.dt.float32
AF = mybir.ActivationFunctionType
ALU = mybir.AluOpType
bf16 = ml_dtypes.bfloat16

V, E = 50000, 300
HW_, HS_ = 256, 256
NCLS = 10
B, S, W = 64, 16, 32
NCORES = 8
BC = B // NCORES          # docs per core = 8
NW = BC * S               # word-level batch per core = 128
GW = 3 * HW_              # 768


def _build_program():
    nc = bacc.Bacc(
        "TRN2",
        target_bir_lowering=False,
        debug=False,
        enable_asserts=False,
        num_devices=NCORES,
    )

    # ---- DRAM I/O ----
    # G cols: [r0|r1|z0|z1|n0|n1] (256 each); r/z include bhh, n excludes bhh_n
    G_d = nc.dram_tensor("G", [V, 1536], BF, kind="ExternalInput")
    toks_d = nc.dram_tensor("toks", [128, 32], mybir.dt.int32, kind="ExternalInput")
    whhT_d = nc.dram_tensor("whhT", [4, 128, GW], BF, kind="ExternalInput")
    brow_d = nc.dram_tensor("brow", [1, 512], BF, kind="ExternalInput")
    waT_d = nc.dram_tensor("waT", [512, 512], BF, kind="ExternalInput")
    barow_d = nc.dram_tensor("barow", [1, 512], BF, kind="ExternalInput")
    vcol_d = nc.dram_tensor("vcol", [128, 4], BF, kind="ExternalInput")
    # sentence input proj, feature-major lhsT blocks; cols [r0|r1|z0|z1|n0|n1]
    swT_d = nc.dram_tensor("swT", [4, 128, 1536], BF, kind="ExternalInput")
    sbirow_d = nc.dram_tensor("sbirow", [1, 1536], BF, kind="ExternalInput")
    swhhT_d = nc.dram_tensor("swhhT", [4, 128, GW], BF, kind="ExternalInput")
    sbrow_d = nc.dram_tensor("sbrow", [1, 512], BF, kind="ExternalInput")
    sawT_d = nc.dram_tensor("sawT", [512, 512], BF, kind="ExternalInput")
    sbarow_d = nc.dram_tensor("sbarow", [1, 512], BF, kind="ExternalInput")
    svcol_d = nc.dram_tensor("svcol", [128, 4], BF, kind="ExternalInput")
    fcwT_d = nc.dram_tensor("fcwT", [512, NCLS], BF, kind="ExternalInput")
    fcb_d = nc.dram_tensor("fcb", [1, NCLS], BF, kind="ExternalInput")
    out_d = nc.dram_tensor("out", [BC, NCLS], F32, kind="ExternalOutput")

    with tile.TileContext(nc) as tc:
        _body(nc, tc, locals())
    nc.compile()
    return nc


def _body(nc, tc, d):
    G_ap = d["G_d"].ap()
    with tc.tile_pool(name="const", bufs=1) as cp:
        # ---- constants / weights in SBUF ----
        toks = cp.tile([128, 32], mybir.dt.int32)
        nc.sync.dma_start(out=toks, in_=d["toks_d"].ap())
        ident = cp.tile([128, 128], BF)
        make_identity(nc, ident)
        ident32 = cp.tile([16, 16], F32)
        make_identity(nc, ident32)
        ones = cp.tile([1, 128], BF)
        nc.gpsimd.memset(ones, 1.0)

        whh = cp.tile([128, 4 * GW], BF)  # blocks (d,k); cols [r|z|n] per block
        for j in range(4):
            nc.sync.dma_start(out=whh[:, j * GW:(j + 1) * GW],
                              in_=d["whhT_d"].ap()[j])
        brow = cp.tile([1, 512], BF)      # [bhh0_n | bhh1_n]
        nc.sync.dma_start(out=brow, in_=d["brow_d"].ap())
        waT = cp.tile([128, 4 * 512], BF)  # k-chunks of wa_W.T
        for j in range(4):
            nc.sync.dma_start(out=waT[:, j * 512:(j + 1) * 512],
                              in_=d["waT_d"].ap()[j * 128:(j + 1) * 128, :])
        barow = cp.tile([1, 512], BF)
        nc.sync.dma_start(out=barow, in_=d["barow_d"].ap())
        vcol = cp.tile([128, 4], BF)
        nc.sync.dma_start(out=vcol, in_=d["vcol_d"].ap())

        swT = cp.tile([128, 4 * 1536], BF)
        for j in range(4):
            nc.sync.dma_start(out=swT[:, j * 1536:(j + 1) * 1536],
                              in_=d["swT_d"].ap()[j])
        sbirow = cp.tile([1, 1536], BF)
        nc.sync.dma_start(out=sbirow, in_=d["sbirow_d"].ap())
        swhh = cp.tile([128, 4 * GW], BF)
        for j in range(4):
            nc.sync.dma_start(out=swhh[:, j * GW:(j + 1) * GW],
                              in_=d["swhhT_d"].ap()[j])
        sbrow = cp.tile([1, 512], BF)
        nc.sync.dma_start(out=sbrow, in_=d["sbrow_d"].ap())
        sawT = cp.tile([128, 4 * 512], BF)
        for j in range(4):
            nc.sync.dma_start(out=sawT[:, j * 512:(j + 1) * 512],
                              in_=d["sawT_d"].ap()[j * 128:(j + 1) * 128, :])
        sbarow = cp.tile([1, 512], BF)
        nc.sync.dma_start(out=sbarow, in_=d["sbarow_d"].ap())
        svcol = cp.tile([128, 4], BF)
        nc.sync.dma_start(out=svcol, in_=d["svcol_d"].ap())
        fcwT = cp.tile([128, 4 * NCLS], BF)
        for j in range(4):
            nc.sync.dma_start(out=fcwT[:, j * NCLS:(j + 1) * NCLS],
                              in_=d["fcwT_d"].ap()[j * 128:(j + 1) * 128, :])
        fcb = cp.tile([1, NCLS], BF)
        nc.sync.dma_start(out=fcb, in_=d["fcb_d"].ap())

        # ---- persistent state ----
        # per-direction h history (separate tiles so the two GRU chains
        # decouple in the tile-granular dependency tracker)
        hist_a = cp.tile([128, 33 * 256], BF)
        hist_b = cp.tile([128, 33 * 256], BF)
        hist = [hist_a, hist_b]
        nc.gpsimd.memset(hist[0][:, 0:256], 0.0)
        nc.gpsimd.memset(hist[1][:, 0:256], 0.0)
        hT0a = cp.tile([128, 256], BF)           # transposed h, step -1, dir 0
        nc.gpsimd.memset(hT0a, 0.0)
        hT0b = cp.tile([128, 256], BF)
        nc.gpsimd.memset(hT0b, 0.0)
        sent = cp.tile([128, 512], BF)           # word-attn out, feature-major
        gisT = cp.tile([128, 1536], BF)          # sentence gi, feature-major
        hs = cp.tile([128, 32], BF)              # sentence h state (in-place)
        nc.gpsimd.memset(hs, 0.0)
        hstok = cp.tile([8, 16 * 512], BF)       # sentence h history, tok-major
        aw = cp.tile([128, 32], F32)             # word attn weights
        doc_sb = cp.tile([128, 32], BF)          # doc vectors, feature-major

        def u_proj(wp, pup, hfa, hfb):
            """attention u for feature-major state (hfa, hfb)."""
            pu = pup.tile([128, 512], F32, tag="pu")
            chunks = (hfa[:, 0:128], hfa[:, 128:256],
                      hfb[:, 0:128], hfb[:, 128:256])
            for jc in range(4):
                nc.tensor.matmul(pu[:, jc * 128:(jc + 1) * 128],
                                 lhsT=barow[0:1, jc * 128:(jc + 1) * 128],
                                 rhs=ones, start=(jc == 0), stop=False)
                for c in range(4):
                    nc.tensor.matmul(
                        pu[:, jc * 128:(jc + 1) * 128],
                        lhsT=waT[:, c * 512 + jc * 128:c * 512 + (jc + 1) * 128],
                        rhs=chunks[c],
                        start=False, stop=(jc == 3 and c == 3))
            u = wp.tile([128, 512], BF, tag="u")
            nc.scalar.activation(u, pu, AF.Tanh)
            return u

        def u_score(psc, u, col):
            for jc in range(4):
                nc.tensor.matmul(psc[:, col:col + 1],
                                 lhsT=u[:, jc * 128:(jc + 1) * 128],
                                 rhs=vcol[:, jc:jc + 1],
                                 start=(jc == 0), stop=(jc == 3))

        # ================= word stage =================
        # one psum TILE per bank: the dependency tracker is tile-granular,
        # so shared tiles would serialize the two direction chains.
        # Word gate math runs FEATURE-major (gates/features on partitions,
        # tokens on the free dim): h_new lands in SBUF as the next step's
        # matmul rhs directly -- no transpose/copy on the serial chain. The
        # token-major history for the attention weighted sum is built by
        # off-chain transposes+copies into hist[].
        with tc.tile_pool(name="wp", bufs=2) as wp, \
             tc.tile_pool(name="wgi", bufs=4) as wgi, \
             tc.tile_pool(name="pg", bufs=1, space="PSUM") as pgp, \
             tc.tile_pool(name="pu", bufs=1, space="PSUM") as pup, \
             tc.tile_pool(name="psc", bufs=1, space="PSUM") as pscp:
            psc_t = pscp.tile([128, 512], F32, tag="psc")   # bank-padded
            psc = psc_t[:, 0:32]
            hf = (hT0a, hT0b)       # feature-major state [f, (k,tok)]
            pending = None          # (u_tile, score col) awaiting score mms
            for t in range(32):
                gi = wgi.tile([128, 1536], BF, tag="gi")
                nc.gpsimd.indirect_dma_start(
                    out=gi[:, :], out_offset=None, in_=G_ap[:, :],
                    in_offset=bass.IndirectOffsetOnAxis(ap=toks[:, t:t + 1], axis=0),
                )
                pr0 = pgp.tile([128, 512], F32, tag="pr0")
                pr1 = pgp.tile([128, 512], F32, tag="pr1")
                pn0 = pgp.tile([128, 512], F32, tag="pn0")
                pn1 = pgp.tile([128, 512], F32, tag="pn1")
                ginb = pgp.tile([128, 512], BF, tag="ginb")
                ptw = pgp.tile([128, 512], BF, tag="ptw")
                pr = [pr0, pr1]                           # [r | z] per dir
                pn = [pn0[:, 0:256], pn1[:, 0:256]]
                pzs = [pr0[:, 256:512], pr1[:, 256:512]]
                gin = [ginb[:, 0:256], ginb[:, 256:512]]
                # inject gi feature-major (transpose via regular matmul with
                # identity rhs; fp32 psum out). No h dependency -> runs early.
                for dd in range(2):
                    for gc in range(2):
                        nc.tensor.matmul(
                            pr[dd][:, gc * 128:(gc + 1) * 128],
                            lhsT=gi[:, dd * 256 + gc * 128:dd * 256 + (gc + 1) * 128],
                            rhs=ident, start=(gc == 0), stop=False)
                        nc.tensor.matmul(
                            pn[dd][:, gc * 128:(gc + 1) * 128],
                            lhsT=brow[0:1, dd * 256 + gc * 128:dd * 256 + (gc + 1) * 128],
                            rhs=ones, start=(gc == 0), stop=False)
                        nc.tensor.transpose(
                            gin[dd][:, gc * 128:(gc + 1) * 128],
                            in_=gi[:, 1024 + dd * 256 + gc * 128:
                                   1024 + dd * 256 + (gc + 1) * 128],
                            identity=ident)
                        nc.tensor.matmul(
                            pzs[dd][:, gc * 128:(gc + 1) * 128],
                            lhsT=gi[:, 512 + dd * 256 + gc * 128:
                                    512 + dd * 256 + (gc + 1) * 128],
                            rhs=ident, start=False, stop=False)
                # recurrent, chain-priority order per dir: r (pr bank,
                # stop), n then z (pn bank, stop on z's last)
                for dd in range(2):
                    for gc in range(2):
                        for k in range(2):
                            w = whh[:, (dd * 2 + k) * GW:(dd * 2 + k + 1) * GW]
                            nc.tensor.matmul(
                                pr[dd][:, gc * 128:(gc + 1) * 128],
                                lhsT=w[:, gc * 128:(gc + 1) * 128],
                                rhs=hf[dd][:, k * 128:(k + 1) * 128],
                                start=False, stop=False)
                    for gc in range(2):
                        for k in range(2):
                            w = whh[:, (dd * 2 + k) * GW:(dd * 2 + k + 1) * GW]
                            nc.tensor.matmul(
                                pzs[dd][:, gc * 128:(gc + 1) * 128],
                                lhsT=w[:, 256 + gc * 128:256 + (gc + 1) * 128],
                                rhs=hf[dd][:, k * 128:(k + 1) * 128],
                                start=False, stop=(gc == 1 and k == 1))
                    for gc in range(2):
                        for k in range(2):
                            w = whh[:, (dd * 2 + k) * GW:(dd * 2 + k + 1) * GW]
                            nc.tensor.matmul(
                                pn[dd][:, gc * 128:(gc + 1) * 128],
                                lhsT=w[:, 512 + gc * 128:512 + (gc + 1) * 128],
                                rhs=hf[dd][:, k * 128:(k + 1) * 128],
                                start=False, stop=(gc == 1 and k == 1))
                # scores for the u finished last step
                if pending is not None:
                    u_score(psc, pending[0], pending[1])

                # gate math: Act order [rs0, rs1, nn0, nn1, zs, u-tanh]
                rs0 = wp.tile([128, 256], BF, tag="rs0")
                nc.scalar.activation(rs0, pr0[:, 0:256], AF.Sigmoid)
                rs1 = wp.tile([128, 256], BF, tag="rs1")
                nc.scalar.activation(rs1, pr1[:, 0:256], AF.Sigmoid)
                t1_0 = wp.tile([128, 256], BF, tag="t10")
                nc.vector.tensor_tensor(t1_0, rs0, pn[0], op=ALU.mult)
                np_0 = wp.tile([128, 256], BF, tag="np0")
                nc.vector.tensor_add(np_0, t1_0, gin[0])
                t1_1 = wp.tile([128, 256], BF, tag="t11")
                nc.vector.tensor_tensor(t1_1, rs1, pn[1], op=ALU.mult)
                np_1 = wp.tile([128, 256], BF, tag="np1")
                nc.vector.tensor_add(np_1, t1_1, gin[1])
                zs0 = wp.tile([128, 256], BF, tag="zs0")
                nc.scalar.activation(zs0, pzs[0], AF.Sigmoid)
                zs1 = wp.tile([128, 256], BF, tag="zs1")
                nc.scalar.activation(zs1, pzs[1], AF.Sigmoid)
                zsl = [zs0, zs1]
                nn0 = wp.tile([128, 256], BF, tag="nn0")
                nc.scalar.activation(nn0, np_0, AF.Tanh)
                nn1 = wp.tile([128, 256], BF, tag="nn1")
                nc.scalar.activation(nn1, np_1, AF.Tanh)
                # u for h_{t-1} -- issued here so the chain acts stay ahead
                # of u-tanh in the Act FIFO
                if t > 0:
                    pending = (u_proj(wp, pup, hf[0], hf[1]), t - 1)
                hfnew = []
                for dd, nn in ((0, nn0), (1, nn1)):
                    dv = wp.tile([128, 256], BF, tag=f"dv{dd}")
                    nc.vector.tensor_sub(dv, hf[dd], nn)
                    zd = wp.tile([128, 256], BF, tag=f"zd{dd}")
                    nc.vector.tensor_tensor(zd, zsl[dd], dv, op=ALU.mult)
                    hfd = wp.tile([128, 256], BF, tag=f"hf{dd}")
                    nc.vector.tensor_add(hfd, nn, zd)
                    hfnew.append(hfd)
                # off-chain: token-major history for the weighted sum
                for dd in range(2):
                    for k in range(2):
                        nc.tensor.transpose(
                            ptw[:, (dd * 2 + k) * 128:(dd * 2 + k + 1) * 128],
                            in_=hfnew[dd][:, k * 128:(k + 1) * 128],
                            identity=ident)
                for dd in range(2):
                    nc.vector.tensor_copy(
                        hist[dd][:, (t + 1) * 256:(t + 2) * 256],
                        ptw[:, dd * 256:(dd + 1) * 256])
                hf = (hfnew[0], hfnew[1])

            # epilogue: flush attention pipeline (h_30, h_31)
            u_score(psc, pending[0], pending[1])
            u31 = u_proj(wp, pup, hf[0], hf[1])
            u_score(psc, u31, 31)

            # ---- word softmax ----
            nmx = wp.tile([128, 1], F32, tag="nmx")
            nc.vector.tensor_reduce(nmx, psc, axis=mybir.AxisListType.X,
                                    op=ALU.max, negate=True)
            ew = wp.tile([128, 32], F32, tag="ew")
            se = wp.tile([128, 1], F32, tag="se")
            nc.scalar.activation(ew, psc, AF.Exp, bias=nmx, accum_out=se)
            rse = wp.tile([128, 1], F32, tag="rse")
            nc.vector.reciprocal(rse, se)
            nc.vector.tensor_scalar_mul(aw, ew, rse)

        # ---- weighted sum (diag matmuls) + sentence input projection ----
        with tc.tile_pool(name="mid", bufs=8) as mp, \
             tc.tile_pool(name="pws", bufs=1, space="PSUM") as pwsp, \
             tc.tile_pool(name="pgs", bufs=1, space="PSUM") as pgsp:
            pws = pwsp.tile([128, 512], F32, tag="pws")
            for t in range(32):
                dg = mp.tile([128, 128], BF, tag="dg")
                nc.vector.tensor_scalar_mul(dg, ident, aw[:, t:t + 1])
                for c in range(4):
                    hsl = hist[c // 2][:, (t + 1) * 256 + (c % 2) * 128:
                                       (t + 1) * 256 + (c % 2 + 1) * 128]
                    nc.tensor.matmul(pws[:, c * 128:(c + 1) * 128],
                                     lhsT=hsl,
                                     rhs=dg, start=(t == 0 and c == 0),
                                     stop=(t == 31 and c == 3))
            nc.scalar.copy(sent[:, 0:256], pws[:, 0:256])
            nc.vector.tensor_copy(sent[:, 256:512], pws[:, 256:512])

            # gi_s = SWih @ sent + biases, feature-major [g, p]
            pgs = pgsp.tile([128, 1536], F32, tag="pgs")
            for gc in range(12):
                sl = slice(gc * 128, (gc + 1) * 128)
                nc.tensor.matmul(pgs[:, sl], lhsT=sbirow[0:1, sl], rhs=ones,
                                 start=(gc % 4 == 0), stop=False)
                for k in range(4):
                    nc.tensor.matmul(
                        pgs[:, sl],
                        lhsT=swT[:, k * 1536 + gc * 128:k * 1536 + (gc + 1) * 128],
                        rhs=sent[:, k * 128:(k + 1) * 128],
                        start=False, stop=(gc % 4 == 3 and k == 3))
            nc.scalar.copy(gisT[:, 0:768], pgs[:, 0:768])
            nc.vector.tensor_copy(gisT[:, 768:1536], pgs[:, 768:1536])

        # ================= sentence stage (feature-major, batch 8) ==========
        gisT_r = gisT.rearrange("p (c x) -> p c x", c=12)
        with tc.tile_pool(name="sp", bufs=2) as sp, \
             tc.tile_pool(name="pgss", bufs=1, space="PSUM") as pgssp, \
             tc.tile_pool(name="pus", bufs=1, space="PSUM") as pusp, \
             tc.tile_pool(name="ptx", bufs=2, space="PSUM") as ptxp, \
             tc.tile_pool(name="pscs", bufs=1, space="PSUM") as pscsp:
            pscs_t = pscsp.tile([8, 512], F32, tag="pscs")     # bank-padded
            pscs = pscs_t[:, 0:16]
            spending = None
            for s in range(16):
                przs_t = pgssp.tile([128, 512], F32, tag="przs")  # bank-padded
                pns_t = pgssp.tile([128, 512], F32, tag="pns")    # bank-padded
                przs = przs_t[:, 0:64]
                pns = pns_t[:, 0:32]
                # inject gi_s (r chunks 0-3, z chunks 4-7) + n bias
                for c in range(8):
                    nc.tensor.matmul(przs[:, c * 8:(c + 1) * 8], lhsT=ident,
                                     rhs=gisT_r[:, c, 8 * s:8 * s + 8],
                                     start=(c == 0), stop=False)
                for c in range(4):
                    nc.tensor.matmul(pns[:, c * 8:(c + 1) * 8],
                                     lhsT=sbrow[0:1, c * 128:(c + 1) * 128],
                                     rhs=ones[:, 0:8], start=(c == 0), stop=False)
                # recurrent (r, n, z); stop only on last mm per bank
                for gsel, goff in ((0, 0), (1, 512), (2, 256)):   # r, n, z
                    for dd in range(2):
                        for gc in range(2):
                            for k in range(2):
                                last = (dd == 1 and gc == 1 and k == 1)
                                lhs = swhh[:, (dd * 2 + k) * GW + goff
                                           + gc * 128:(dd * 2 + k) * GW
                                           + goff + (gc + 1) * 128]
                                rh = hs[:, (dd * 2 + k) * 8:(dd * 2 + k + 1) * 8]
                                if gsel == 0:
                                    out = przs[:, (dd * 2 + gc) * 8:(dd * 2 + gc + 1) * 8]
                                    st = False
                                elif gsel == 1:
                                    out = pns[:, (dd * 2 + gc) * 8:(dd * 2 + gc + 1) * 8]
                                    st = last
                                else:
                                    out = przs[:, 32 + (dd * 2 + gc) * 8:
                                               32 + (dd * 2 + gc + 1) * 8]
                                    st = last
                                nc.tensor.matmul(out, lhsT=lhs, rhs=rh,
                                                 start=False, stop=st)
                if spending is not None:
                    us_, col = spending
                    for jc in range(4):
                        nc.tensor.matmul(pscs[:, col:col + 1],
                                         lhsT=us_[:, jc * 8:(jc + 1) * 8],
                                         rhs=svcol[:, jc:jc + 1],
                                         start=(jc == 0), stop=(jc == 3))
                # gate math (fused dirs; ops are [128, 32])
                rs = sp.tile([128, 32], BF, tag="rs")
                nc.scalar.activation(rs, przs[:, 0:32], AF.Sigmoid)
                t1 = sp.tile([128, 32], BF, tag="t1")
                nc.vector.tensor_tensor(t1, rs, pns, op=ALU.mult)
                npre = sp.tile([128, 32], BF, tag="np")
                nc.vector.tensor_add(npre.rearrange("p (c j) -> p c j", c=4),
                                     t1.rearrange("p (c j) -> p c j", c=4),
                                     gisT_r[:, 8:12, 8 * s:8 * s + 8])
                nn = sp.tile([128, 32], BF, tag="nn")
                nc.scalar.activation(nn, npre, AF.Tanh)
                zs = sp.tile([128, 32], BF, tag="zs")
                nc.scalar.activation(zs, przs[:, 32:64], AF.Sigmoid)
                dv = sp.tile([128, 32], BF, tag="dv")
                nc.vector.tensor_sub(dv, hs, nn)
                zd = sp.tile([128, 32], BF, tag="zd")
                nc.vector.tensor_tensor(zd, zs, dv, op=ALU.mult)
                nc.vector.tensor_add(hs, nn, zd)
                # h history (token-major): transpose to base-0 psum, copy out
                ptx_t = ptxp.tile([8, 1024], BF, tag="ptx")    # bank-padded
                ptx = ptx_t[:, 0:512]
                for c in range(4):
                    nc.tensor.transpose(ptx[:, c * 128:(c + 1) * 128],
                                        in_=hs[:, c * 8:(c + 1) * 8],
                                        identity=ident)
                hcp = (s, ptx)   # hstok copy deferred below (Act, post-u)
                # attention u for this step's h
                pus_t = pusp.tile([128, 512], F32, tag="pus")  # bank-padded
                pus = pus_t[:, 0:32]
                for jc in range(4):
                    nc.tensor.matmul(pus[:, jc * 8:(jc + 1) * 8],
                                     lhsT=sbarow[0:1, jc * 128:(jc + 1) * 128],
                                     rhs=ones[:, 0:8], start=(jc == 0), stop=False)
                    for c in range(4):
                        nc.tensor.matmul(
                            pus[:, jc * 8:(jc + 1) * 8],
                            lhsT=sawT[:, c * 512 + jc * 128:c * 512 + (jc + 1) * 128],
                            rhs=hs[:, c * 8:(c + 1) * 8],
                            start=False, stop=(jc == 3 and c == 3))
                us = sp.tile([128, 32], BF, tag="us")
                nc.scalar.activation(us, pus, AF.Tanh)
                spending = (us, s)
                nc.vector.tensor_copy(hstok[:, s * 512:s * 512 + 256],
                                      hcp[1][:, 0:256])
                nc.vector.tensor_copy(hstok[:, s * 512 + 256:s * 512 + 512],
                                      hcp[1][:, 256:512])
            us_, col = spending
            for jc in range(4):
                nc.tensor.matmul(pscs[:, col:col + 1],
                                 lhsT=us_[:, jc * 8:(jc + 1) * 8],
                                 rhs=svcol[:, jc:jc + 1],
                                 start=(jc == 0), stop=(jc == 3))

            # sentence softmax + weighted sum + classifier + log_softmax
            nmx = sp.tile([8, 1], F32, tag="snmx")
            nc.vector.tensor_reduce(nmx, pscs, axis=mybir.AxisListType.X,
                                    op=ALU.max, negate=True)
            ew = sp.tile([8, 16], F32, tag="sew")
            se = sp.tile([8, 1], F32, tag="sse")
            nc.scalar.activation(ew, pscs, AF.Exp, bias=nmx, accum_out=se)
            rse = sp.tile([8, 1], F32, tag="srse")
            nc.vector.reciprocal(rse, se)
            aws = sp.tile([8, 16], F32, tag="saw")
            nc.vector.tensor_scalar_mul(aws, ew, rse)
            pdoc_t = pgssp.tile([128, 512], F32, tag="przs")  # reuse przs bank
            pdoc = pdoc_t[:, 0:32]
            for s in range(16):
                dg = sp.tile([8, 8], BF, tag=f"sdg{s % 8}")
                nc.vector.tensor_scalar_mul(dg, ident[0:8, 0:8], aws[:, s:s + 1])
                for c in range(4):
                    nc.tensor.matmul(pdoc[:, c * 8:(c + 1) * 8],
                                     lhsT=hstok[:, s * 512 + c * 128:s * 512 + (c + 1) * 128],
                                     rhs=dg, start=(s == 0 and c == 0),
                                     stop=(s == 15 and c == 3))
            nc.vector.tensor_copy(doc_sb, pdoc)
            pcl_t = pgssp.tile([128, 512], F32, tag="pns")    # reuse pns bank
            pl = pcl_t[0:10, 0:8]
            plt = pcl_t[0:8, 16:26]
            nc.tensor.matmul(pl, lhsT=fcb[0:1, :], rhs=ones[:, 0:8],
                             start=True, stop=False)
            for c in range(4):
                nc.tensor.matmul(pl, lhsT=fcwT[:, c * NCLS:(c + 1) * NCLS],
                                 rhs=doc_sb[:, c * 8:(c + 1) * 8],
                                 start=False, stop=(c == 3))
            lg = sp.tile([10, 8], F32, tag="lg")
            nc.vector.tensor_copy(lg, pl)
            nc.tensor.transpose(plt, in_=lg, identity=ident32[0:10, 0:10])
            nmx2 = sp.tile([8, 1], F32, tag="nmx2")
            nc.vector.tensor_reduce(nmx2, plt, axis=mybir.AxisListType.X,
                                    op=ALU.max, negate=True)
            e2 = sp.tile([8, NCLS], F32, tag="e2")
            se2 = sp.tile([8, 1], F32, tag="se2")
            nc.scalar.activation(e2, plt, AF.Exp, bias=nmx2, accum_out=se2)
            lse = sp.tile([8, 1], F32, tag="lse")
            nc.scalar.activation(lse, se2, AF.Ln)
            out_sb = sp.tile([8, NCLS], F32, tag="out_sb")
            nc.vector.tensor_scalar(out=out_sb, in0=plt, scalar1=nmx2,
                                    scalar2=lse, op0=ALU.add, op1=ALU.subtract)
            nc.sync.dma_start(out=d["out_d"].ap(), in_=out_sb)


# ---------------------------------------------------------------------------
# host side
# ---------------------------------------------------------------------------

def _prep_inputs(inputs):
    """Build the per-core in_maps (host preprocessing + sharding)."""
    f32 = np.float32
    emb = np.asarray(inputs["emb"], f32)
    w_Wih = np.asarray(inputs["w_Wih"], f32)
    w_Whh = np.asarray(inputs["w_Whh"], f32)
    w_bih = np.asarray(inputs["w_bih"], f32)
    w_bhh = np.asarray(inputs["w_bhh"], f32)
    wa_W = np.asarray(inputs["wa_W"], f32)
    wa_b = np.asarray(inputs["wa_b"], f32)
    wa_v = np.asarray(inputs["wa_v"], f32)
    s_Wih = np.asarray(inputs["s_Wih"], f32)
    s_Whh = np.asarray(inputs["s_Whh"], f32)
    s_bih = np.asarray(inputs["s_bih"], f32)
    s_bhh = np.asarray(inputs["s_bhh"], f32)
    sa_W = np.asarray(inputs["sa_W"], f32)
    sa_b = np.asarray(inputs["sa_b"], f32)
    sa_v = np.asarray(inputs["sa_v"], f32)
    fc_W = np.asarray(inputs["fc_W"], f32)
    fc_b = np.asarray(inputs["fc_b"], f32)
    tokens = np.asarray(inputs["tokens"])

    def b(x):
        return np.ascontiguousarray(x.astype(bf16))

    # folded gather table G [V, 1536], cols [r0|r1|z0|z1|n0|n1]
    g0 = emb @ w_Wih[0].T + w_bih[0]
    g0[:, :512] += w_bhh[0][:512]
    g1 = emb @ w_Wih[1].T + w_bih[1]
    g1[:, :512] += w_bhh[1][:512]
    G = np.concatenate([g0[:, 0:256], g1[:, 0:256],
                        g0[:, 256:512], g1[:, 256:512],
                        g0[:, 512:768], g1[:, 512:768]], 1)

    whhT = np.stack([w_Whh[0].T[:128], w_Whh[0].T[128:],
                     w_Whh[1].T[:128], w_Whh[1].T[128:]])  # [4,128,768]
    brow = np.concatenate([w_bhh[0][512:], w_bhh[1][512:]])[None, :]
    vcol = np.ascontiguousarray(wa_v.reshape(4, 128).T)     # [128, 4]

    # sentence input proj: swihT [512, 1536] cols [r0|r1|z0|z1|n0|n1]
    sg0 = s_Wih[0].T  # [512, 768]
    sg1 = s_Wih[1].T
    swihT = np.concatenate([sg0[:, 0:256], sg1[:, 0:256],
                            sg0[:, 256:512], sg1[:, 256:512],
                            sg0[:, 512:768], sg1[:, 512:768]], 1)
    swT = np.stack([swihT[k * 128:(k + 1) * 128] for k in range(4)])
    sb0 = s_bih[0] + s_bhh[0]
    sb1 = s_bih[1] + s_bhh[1]
    sbirow = np.concatenate([
        sb0[0:256], sb1[0:256], sb0[256:512], sb1[256:512],
        s_bih[0][512:768], s_bih[1][512:768]])[None, :]
    swhhT = np.stack([s_Whh[0].T[:128], s_Whh[0].T[128:],
                      s_Whh[1].T[:128], s_Whh[1].T[128:]])
    sbrow = np.concatenate([s_bhh[0][512:], s_bhh[1][512:]])[None, :]
    svcol = np.ascontiguousarray(sa_v.reshape(4, 128).T)

    shared = {
        "G": b(G), "whhT": b(whhT), "brow": b(brow),
        "waT": b(wa_W.T), "barow": b(wa_b[None, :]), "vcol": b(vcol),
        "swT": b(swT), "sbirow": b(sbirow), "swhhT": b(swhhT),
        "sbrow": b(sbrow), "sawT": b(sa_W.T), "sbarow": b(sa_b[None, :]),
        "svcol": b(svcol), "fcwT": b(fc_W.T), "fcb": b(fc_b[None, :]),
    }
    in_maps = []
    for c in range(NCORES):
        # word-row p = s*8 + doc
        tk = np.ascontiguousarray(
            np.transpose(tokens[c * BC:(c + 1) * BC], (1, 0, 2))
            .reshape(NW, W).astype(np.int32))
        in_maps.append({**shared, "toks": tk})
    return in_maps


_NC_CACHE = {}


def _get_nc():
    if "nc" not in _NC_CACHE:
        _NC_CACHE["nc"] = _build_program()
    return _NC_CACHE["nc"]


def kernel(**inputs) -> np.ndarray:
    nc = _get_nc()
    in_maps = _prep_inputs(inputs)
    res = bass_utils.run_bass_kernel_spmd(nc, in_maps, core_ids=list(range(NCORES)))
    outs = []
    for c in range(NCORES):
        o = np.asarray(res.results[c]["out"], np.float32)
        outs.append(o)
    return np.concatenate(outs, 0)


# revision 4
# speedup vs baseline: 1.0252x; 1.0252x over previous
"""HAN forward pass on 8 TRN2 NeuronCores — v2 (latency-optimized).

Data-parallel over batch (8 docs/core), no collectives. vs v1:
* r/z sigmoids split; per-direction gate chains interleaved so the two
  independent GRU recurrences pipeline across PE/Act/DVE.
* Attention u-projection emitted feature-major (lhsT=Wa^T blocks,
  rhs=hT) one step behind the recurrence; the score reduction is 4
  tiny PE matmuls into PSUM instead of a 512-wide DVE op.
* Word-attention weighted sum via diag(a_t) matmuls accumulating in
  PSUM (produces sent feature-major for the sentence stage directly).
* Sentence stage fully feature-major: batch-8 matmuls run at N=8
  instead of N=512.
"""

import numpy as np
import ml_dtypes

import concourse.bass as bass
import concourse.mybir as mybir
import concourse.tile as tile
from concourse import bacc, bass_utils
from concourse.masks import make_identity

BF = mybir.dt.bfloat16
F32 = mybir.dt.float32
AF = mybir.ActivationFunctionType
ALU = mybir.AluOpType
bf16 = ml_dtypes.bfloat16

V, E = 50000, 300
HW_, HS_ = 256, 256
NCLS = 10
B, S, W = 64, 16, 32
NCORES = 8
BC = B // NCORES          # docs per core = 8
NW = BC * S               # word-level batch per core = 128
GW = 3 * HW_              # 768


def _build_program():
    nc = bacc.Bacc(
        "TRN2",
        target_bir_lowering=False,
        debug=False,
        enable_asserts=False,
        num_devices=NCORES,
    )

    # ---- DRAM I/O ----
    # G cols: [r0|r1|z0|z1|n0|n1] (256 each); r/z include bhh, n excludes bhh_n
    G_d = nc.dram_tensor("G", [V, 1536], BF, kind="ExternalInput")
    toks_d = nc.dram_tensor("toks", [128, 32], mybir.dt.int32, kind="ExternalInput")
    whhT_d = nc.dram_tensor("whhT", [4, 128, GW], BF, kind="ExternalInput")
    brow_d = nc.dram_tensor("brow", [1, 512], BF, kind="ExternalInput")
    waT_d = nc.dram_tensor("waT", [512, 512], BF, kind="ExternalInput")
    barow_d = nc.dram_tensor("barow", [1, 512], BF, kind="ExternalInput")
    vcol_d = nc.dram_tensor("vcol", [128, 4], BF, kind="ExternalInput")
    # sentence input proj, feature-major lhsT blocks; cols [r0|r1|z0|z1|n0|n1]
    swT_d = nc.dram_tensor("swT", [4, 128, 1536], BF, kind="ExternalInput")
    sbirow_d = nc.dram_tensor("sbirow", [1, 1536], BF, kind="ExternalInput")
    swhhT_d = nc.dram_tensor("swhhT", [4, 128, GW], BF, kind="ExternalInput")
    sbrow_d = nc.dram_tensor("sbrow", [1, 512], BF, kind="ExternalInput")
    sawT_d = nc.dram_tensor("sawT", [512, 512], BF, kind="ExternalInput")
    sbarow_d = nc.dram_tensor("sbarow", [1, 512], BF, kind="ExternalInput")
    svcol_d = nc.dram_tensor("svcol", [128, 4], BF, kind="ExternalInput")
    fcwT_d = nc.dram_tensor("fcwT", [512, NCLS], BF, kind="ExternalInput")
    fcb_d = nc.dram_tensor("fcb", [1, NCLS], BF, kind="ExternalInput")
    out_d = nc.dram_tensor("out", [BC, NCLS], F32, kind="ExternalOutput")

    with tile.TileContext(nc) as tc:
        _body(nc, tc, locals())
    nc.compile()
    return nc


def _body(nc, tc, d):
    G_ap = d["G_d"].ap()
    with tc.tile_pool(name="const", bufs=1) as cp:
        # ---- constants / weights in SBUF ----
        toks = cp.tile([128, 32], mybir.dt.int32)
        nc.sync.dma_start(out=toks, in_=d["toks_d"].ap())
        ident = cp.tile([128, 128], BF)
        make_identity(nc, ident)
        ident32 = cp.tile([16, 16], F32)
        make_identity(nc, ident32)
        ones = cp.tile([1, 128], BF)
        nc.gpsimd.memset(ones, 1.0)

        whh = cp.tile([128, 4 * GW], BF)  # blocks (d,k); cols [r|z|n] per block
        for j in range(4):
            nc.sync.dma_start(out=whh[:, j * GW:(j + 1) * GW],
                              in_=d["whhT_d"].ap()[j])
        brow = cp.tile([1, 512], BF)      # [bhh0_n | bhh1_n]
        nc.sync.dma_start(out=brow, in_=d["brow_d"].ap())
        waT = cp.tile([128, 4 * 512], BF)  # k-chunks of wa_W.T
        for j in range(4):
            nc.sync.dma_start(out=waT[:, j * 512:(j + 1) * 512],
                              in_=d["waT_d"].ap()[j * 128:(j + 1) * 128, :])
        barow = cp.tile([1, 512], BF)
        nc.sync.dma_start(out=barow, in_=d["barow_d"].ap())
        vcol = cp.tile([128, 4], BF)
        nc.sync.dma_start(out=vcol, in_=d["vcol_d"].ap())

        swT = cp.tile([128, 4 * 1536], BF)
        for j in range(4):
            nc.sync.dma_start(out=swT[:, j * 1536:(j + 1) * 1536],
                              in_=d["swT_d"].ap()[j])
        sbirow = cp.tile([1, 1536], BF)
        nc.sync.dma_start(out=sbirow, in_=d["sbirow_d"].ap())
        swhh = cp.tile([128, 4 * GW], BF)
        for j in range(4):
            nc.sync.dma_start(out=swhh[:, j * GW:(j + 1) * GW],
                              in_=d["swhhT_d"].ap()[j])
        sbrow = cp.tile([1, 512], BF)
        nc.sync.dma_start(out=sbrow, in_=d["sbrow_d"].ap())
        sawT = cp.tile([128, 4 * 512], BF)
        for j in range(4):
            nc.sync.dma_start(out=sawT[:, j * 512:(j + 1) * 512],
                              in_=d["sawT_d"].ap()[j * 128:(j + 1) * 128, :])
        sbarow = cp.tile([1, 512], BF)
        nc.sync.dma_start(out=sbarow, in_=d["sbarow_d"].ap())
        svcol = cp.tile([128, 4], BF)
        nc.sync.dma_start(out=svcol, in_=d["svcol_d"].ap())
        fcwT = cp.tile([128, 4 * NCLS], BF)
        for j in range(4):
            nc.sync.dma_start(out=fcwT[:, j * NCLS:(j + 1) * NCLS],
                              in_=d["fcwT_d"].ap()[j * 128:(j + 1) * 128, :])
        fcb = cp.tile([1, NCLS], BF)
        nc.sync.dma_start(out=fcb, in_=d["fcb_d"].ap())

        # ---- persistent state ----
        # per-direction h history (separate tiles so the two GRU chains
        # decouple in the tile-granular dependency tracker)
        hist_a = cp.tile([128, 33 * 256], BF)
        hist_b = cp.tile([128, 33 * 256], BF)
        hist = [hist_a, hist_b]
        nc.gpsimd.memset(hist[0][:, 0:256], 0.0)
        nc.gpsimd.memset(hist[1][:, 0:256], 0.0)
        hT0a = cp.tile([128, 256], BF)           # transposed h, step -1, dir 0
        nc.gpsimd.memset(hT0a, 0.0)
        hT0b = cp.tile([128, 256], BF)
        nc.gpsimd.memset(hT0b, 0.0)
        sent = cp.tile([128, 512], BF)           # word-attn out, feature-major
        gisT = cp.tile([128, 1536], BF)          # sentence gi, feature-major
        hs = cp.tile([128, 32], BF)              # sentence h state (in-place)
        nc.gpsimd.memset(hs, 0.0)
        hstok = cp.tile([8, 16 * 512], BF)       # sentence h history, tok-major
        aw = cp.tile([128, 32], F32)             # word attn weights
        doc_sb = cp.tile([128, 32], BF)          # doc vectors, feature-major

        def u_proj(wp, pup, hfa, hfb):
            """attention u for feature-major state (hfa, hfb)."""
            pu = pup.tile([128, 512], F32, tag="pu")
            chunks = (hfa[:, 0:128], hfa[:, 128:256],
                      hfb[:, 0:128], hfb[:, 128:256])
            for jc in range(4):
                nc.tensor.matmul(pu[:, jc * 128:(jc + 1) * 128],
                                 lhsT=barow[0:1, jc * 128:(jc + 1) * 128],
                                 rhs=ones, start=(jc == 0), stop=False)
                for c in range(4):
                    nc.tensor.matmul(
                        pu[:, jc * 128:(jc + 1) * 128],
                        lhsT=waT[:, c * 512 + jc * 128:c * 512 + (jc + 1) * 128],
                        rhs=chunks[c],
                        start=False, stop=(jc == 3 and c == 3))
            u = wp.tile([128, 512], BF, tag="u")
            nc.scalar.activation(u, pu, AF.Tanh)
            return u

        def u_score(psc, u, col):
            for jc in range(4):
                nc.tensor.matmul(psc[:, col:col + 1],
                                 lhsT=u[:, jc * 128:(jc + 1) * 128],
                                 rhs=vcol[:, jc:jc + 1],
                                 start=(jc == 0), stop=(jc == 3))

        # ================= word stage =================
        # one psum TILE per bank: the dependency tracker is tile-granular,
        # so shared tiles would serialize the two direction chains.
        # Word gate math runs FEATURE-major (gates/features on partitions,
        # tokens on the free dim): h_new lands in SBUF as the next step's
        # matmul rhs directly -- no transpose/copy on the serial chain. The
        # token-major history for the attention weighted sum is built by
        # off-chain transposes+copies into hist[].
        with tc.tile_pool(name="wp", bufs=2) as wp, \
             tc.tile_pool(name="wgi", bufs=4) as wgi, \
             tc.tile_pool(name="pg", bufs=1, space="PSUM") as pgp, \
             tc.tile_pool(name="pu", bufs=1, space="PSUM") as pup, \
             tc.tile_pool(name="psc", bufs=1, space="PSUM") as pscp:
            psc_t = pscp.tile([128, 512], F32, tag="psc")   # bank-padded
            psc = psc_t[:, 0:32]
            hf = (hT0a, hT0b)       # feature-major state [f, (k,tok)]
            pending = None          # (u_tile, score col) awaiting score mms
            for t in range(32):
                gi = wgi.tile([128, 1536], BF, tag="gi")
                nc.gpsimd.indirect_dma_start(
                    out=gi[:, :], out_offset=None, in_=G_ap[:, :],
                    in_offset=bass.IndirectOffsetOnAxis(ap=toks[:, t:t + 1], axis=0),
                )
                pr0 = pgp.tile([128, 512], F32, tag="pr0")
                pr1 = pgp.tile([128, 512], F32, tag="pr1")
                pn0 = pgp.tile([128, 512], F32, tag="pn0")
                pn1 = pgp.tile([128, 512], F32, tag="pn1")
                ginb = pgp.tile([128, 512], BF, tag="ginb")
                ptw = pgp.tile([128, 512], BF, tag="ptw")
                pr = [pr0, pr1]                           # [r | z] per dir
                pn = [pn0[:, 0:256], pn1[:, 0:256]]
                pzs = [pr0[:, 256:512], pr1[:, 256:512]]
                gin = [ginb[:, 0:256], ginb[:, 256:512]]
                # inject gi feature-major (transpose via regular matmul with
                # identity rhs; fp32 psum out). No h dependency -> runs early.
                for dd in range(2):
                    for gc in range(2):
                        nc.tensor.matmul(
                            pr[dd][:, gc * 128:(gc + 1) * 128],
                            lhsT=gi[:, dd * 256 + gc * 128:dd * 256 + (gc + 1) * 128],
                            rhs=ident, start=(gc == 0), stop=False)
                        nc.tensor.matmul(
                            pn[dd][:, gc * 128:(gc + 1) * 128],
                            lhsT=brow[0:1, dd * 256 + gc * 128:dd * 256 + (gc + 1) * 128],
                            rhs=ones, start=(gc == 0), stop=False)
                        nc.tensor.transpose(
                            gin[dd][:, gc * 128:(gc + 1) * 128],
                            in_=gi[:, 1024 + dd * 256 + gc * 128:
                                   1024 + dd * 256 + (gc + 1) * 128],
                            identity=ident)
                        nc.tensor.matmul(
                            pzs[dd][:, gc * 128:(gc + 1) * 128],
                            lhsT=gi[:, 512 + dd * 256 + gc * 128:
                                    512 + dd * 256 + (gc + 1) * 128],
                            rhs=ident, start=False, stop=False)
                # recurrent, chain-priority order per dir: r (pr bank,
                # stop), n then z (pn bank, stop on z's last)
                for dd in range(2):
                    for gc in range(2):
                        for k in range(2):
                            w = whh[:, (dd * 2 + k) * GW:(dd * 2 + k + 1) * GW]
                            nc.tensor.matmul(
                                pr[dd][:, gc * 128:(gc + 1) * 128],
                                lhsT=w[:, gc * 128:(gc + 1) * 128],
                                rhs=hf[dd][:, k * 128:(k + 1) * 128],
                                start=False, stop=False)
                    for gc in range(2):
                        for k in range(2):
                            w = whh[:, (dd * 2 + k) * GW:(dd * 2 + k + 1) * GW]
                            nc.tensor.matmul(
                                pzs[dd][:, gc * 128:(gc + 1) * 128],
                                lhsT=w[:, 256 + gc * 128:256 + (gc + 1) * 128],
                                rhs=hf[dd][:, k * 128:(k + 1) * 128],
                                start=False, stop=(gc == 1 and k == 1))
                    for gc in range(2):
                        for k in range(2):
                            w = whh[:, (dd * 2 + k) * GW:(dd * 2 + k + 1) * GW]
                            nc.tensor.matmul(
                                pn[dd][:, gc * 128:(gc + 1) * 128],
                                lhsT=w[:, 512 + gc * 128:512 + (gc + 1) * 128],
                                rhs=hf[dd][:, k * 128:(k + 1) * 128],
                                start=False, stop=(gc == 1 and k == 1))
                # scores for the u finished last step
                if pending is not None:
                    u_score(psc, pending[0], pending[1])

                # gate math: Act order [rs0, rs1, nn0, nn1, zs, u-tanh]
                rs0 = wp.tile([128, 256], BF, tag="rs0")
                nc.scalar.activation(rs0, pr0[:, 0:256], AF.Sigmoid)
                rs1 = wp.tile([128, 256], BF, tag="rs1")
                nc.scalar.activation(rs1, pr1[:, 0:256], AF.Sigmoid)
                t1_0 = wp.tile([128, 256], BF, tag="t10")
                nc.vector.tensor_tensor(t1_0, rs0, pn[0], op=ALU.mult)
                np_0 = wp.tile([128, 256], BF, tag="np0")
                nc.vector.tensor_add(np_0, t1_0, gin[0])
                t1_1 = wp.tile([128, 256], BF, tag="t11")
                nc.vector.tensor_tensor(t1_1, rs1, pn[1], op=ALU.mult)
                np_1 = wp.tile([128, 256], BF, tag="np1")
                nc.vector.tensor_add(np_1, t1_1, gin[1])
                zs0 = wp.tile([128, 256], BF, tag="zs0")
                nc.scalar.activation(zs0, pzs[0], AF.Sigmoid)
                zs1 = wp.tile([128, 256], BF, tag="zs1")
                nc.scalar.activation(zs1, pzs[1], AF.Sigmoid)
                zsl = [zs0, zs1]
                # off-chain on GPSIMD: pieces of h = (1-z)*nn + z*h_prev
                omz0 = wp.tile([128, 256], BF, tag="omz0")
                nc.gpsimd.tensor_scalar(out=omz0, in0=zs0, scalar1=-1.0,
                                        scalar2=1.0, op0=ALU.mult, op1=ALU.add)
                t2_0 = wp.tile([128, 256], BF, tag="t20")
                nc.gpsimd.tensor_tensor(t2_0, zs0, hf[0], op=ALU.mult)
                omz1 = wp.tile([128, 256], BF, tag="omz1")
                nc.gpsimd.tensor_scalar(out=omz1, in0=zs1, scalar1=-1.0,
                                        scalar2=1.0, op0=ALU.mult, op1=ALU.add)
                t2_1 = wp.tile([128, 256], BF, tag="t21")
                nc.gpsimd.tensor_tensor(t2_1, zs1, hf[1], op=ALU.mult)
                omzl = [omz0, omz1]
                t2l = [t2_0, t2_1]
                nn0 = wp.tile([128, 256], BF, tag="nn0")
                nc.scalar.activation(nn0, np_0, AF.Tanh)
                nn1 = wp.tile([128, 256], BF, tag="nn1")
                nc.scalar.activation(nn1, np_1, AF.Tanh)
                # u for h_{t-1} -- issued here so the chain acts stay ahead
                # of u-tanh in the Act FIFO
                if t > 0:
                    pending = (u_proj(wp, pup, hf[0], hf[1]), t - 1)
                hfnew = []
                for dd, nn in ((0, nn0), (1, nn1)):
                    a1 = wp.tile([128, 256], BF, tag=f"a1{dd}")
                    nc.vector.tensor_tensor(a1, omzl[dd], nn, op=ALU.mult)
                    hfd = wp.tile([128, 256], BF, tag=f"hf{dd}")
                    nc.vector.tensor_add(hfd, a1, t2l[dd])
                    hfnew.append(hfd)
                # off-chain: token-major history for the weighted sum
                for dd in range(2):
                    for k in range(2):
                        nc.tensor.transpose(
                            ptw[:, (dd * 2 + k) * 128:(dd * 2 + k + 1) * 128],
                            in_=hfnew[dd][:, k * 128:(k + 1) * 128],
                            identity=ident)
                for dd in range(2):
                    nc.vector.tensor_copy(
                        hist[dd][:, (t + 1) * 256:(t + 2) * 256],
                        ptw[:, dd * 256:(dd + 1) * 256])
                hf = (hfnew[0], hfnew[1])

            # epilogue: flush attention pipeline (h_30, h_31)
            u_score(psc, pending[0], pending[1])
            u31 = u_proj(wp, pup, hf[0], hf[1])
            u_score(psc, u31, 31)

            # ---- word softmax ----
            nmx = wp.tile([128, 1], F32, tag="nmx")
            nc.vector.tensor_reduce(nmx, psc, axis=mybir.AxisListType.X,
                                    op=ALU.max, negate=True)
            ew = wp.tile([128, 32], F32, tag="ew")
            se = wp.tile([128, 1], F32, tag="se")
            nc.scalar.activation(ew, psc, AF.Exp, bias=nmx, accum_out=se)
            rse = wp.tile([128, 1], F32, tag="rse")
            nc.vector.reciprocal(rse, se)
            nc.vector.tensor_scalar_mul(aw, ew, rse)

        # ---- weighted sum (diag matmuls) + sentence input projection ----
        with tc.tile_pool(name="mid", bufs=8) as mp, \
             tc.tile_pool(name="pws", bufs=1, space="PSUM") as pwsp, \
             tc.tile_pool(name="pgs", bufs=1, space="PSUM") as pgsp:
            pws = pwsp.tile([128, 512], F32, tag="pws")
            for t in range(32):
                dg = mp.tile([128, 128], BF, tag="dg")
                nc.vector.tensor_scalar_mul(dg, ident, aw[:, t:t + 1])
                for c in range(4):
                    hsl = hist[c // 2][:, (t + 1) * 256 + (c % 2) * 128:
                                       (t + 1) * 256 + (c % 2 + 1) * 128]
                    nc.tensor.matmul(pws[:, c * 128:(c + 1) * 128],
                                     lhsT=hsl,
                                     rhs=dg, start=(t == 0 and c == 0),
                                     stop=(t == 31 and c == 3))
            nc.scalar.copy(sent[:, 0:256], pws[:, 0:256])
            nc.vector.tensor_copy(sent[:, 256:512], pws[:, 256:512])

            # gi_s = SWih @ sent + biases, feature-major [g, p]
            pgs = pgsp.tile([128, 1536], F32, tag="pgs")
            for gc in range(12):
                sl = slice(gc * 128, (gc + 1) * 128)
                nc.tensor.matmul(pgs[:, sl], lhsT=sbirow[0:1, sl], rhs=ones,
                                 start=(gc % 4 == 0), stop=False)
                for k in range(4):
                    nc.tensor.matmul(
                        pgs[:, sl],
                        lhsT=swT[:, k * 1536 + gc * 128:k * 1536 + (gc + 1) * 128],
                        rhs=sent[:, k * 128:(k + 1) * 128],
                        start=False, stop=(gc % 4 == 3 and k == 3))
            nc.scalar.copy(gisT[:, 0:768], pgs[:, 0:768])
            nc.vector.tensor_copy(gisT[:, 768:1536], pgs[:, 768:1536])

        # ================= sentence stage (feature-major, batch 8) ==========
        gisT_r = gisT.rearrange("p (c x) -> p c x", c=12)
        with tc.tile_pool(name="sp", bufs=2) as sp, \
             tc.tile_pool(name="pgss", bufs=1, space="PSUM") as pgssp, \
             tc.tile_pool(name="pus", bufs=1, space="PSUM") as pusp, \
             tc.tile_pool(name="ptx", bufs=2, space="PSUM") as ptxp, \
             tc.tile_pool(name="pscs", bufs=1, space="PSUM") as pscsp:
            pscs_t = pscsp.tile([8, 512], F32, tag="pscs")     # bank-padded
            pscs = pscs_t[:, 0:16]
            spending = None
            for s in range(16):
                przs_t = pgssp.tile([128, 512], F32, tag="przs")  # bank-padded
                pns_t = pgssp.tile([128, 512], F32, tag="pns")    # bank-padded
                przs = przs_t[:, 0:64]
                pns = pns_t[:, 0:32]
                # inject gi_s (r chunks 0-3, z chunks 4-7) + n bias
                for c in range(8):
                    nc.tensor.matmul(przs[:, c * 8:(c + 1) * 8], lhsT=ident,
                                     rhs=gisT_r[:, c, 8 * s:8 * s + 8],
                                     start=(c == 0), stop=False)
                for c in range(4):
                    nc.tensor.matmul(pns[:, c * 8:(c + 1) * 8],
                                     lhsT=sbrow[0:1, c * 128:(c + 1) * 128],
                                     rhs=ones[:, 0:8], start=(c == 0), stop=False)
                # recurrent (r, n, z); stop only on last mm per bank
                for gsel, goff in ((0, 0), (1, 512), (2, 256)):   # r, n, z
                    for dd in range(2):
                        for gc in range(2):
                            for k in range(2):
                                last = (dd == 1 and gc == 1 and k == 1)
                                lhs = swhh[:, (dd * 2 + k) * GW + goff
                                           + gc * 128:(dd * 2 + k) * GW
                                           + goff + (gc + 1) * 128]
                                rh = hs[:, (dd * 2 + k) * 8:(dd * 2 + k + 1) * 8]
                                if gsel == 0:
                                    out = przs[:, (dd * 2 + gc) * 8:(dd * 2 + gc + 1) * 8]
                                    st = False
                                elif gsel == 1:
                                    out = pns[:, (dd * 2 + gc) * 8:(dd * 2 + gc + 1) * 8]
                                    st = last
                                else:
                                    out = przs[:, 32 + (dd * 2 + gc) * 8:
                                               32 + (dd * 2 + gc + 1) * 8]
                                    st = last
                                nc.tensor.matmul(out, lhsT=lhs, rhs=rh,
                                                 start=False, stop=st)
                if spending is not None:
                    us_, col = spending
                    for jc in range(4):
                        nc.tensor.matmul(pscs[:, col:col + 1],
                                         lhsT=us_[:, jc * 8:(jc + 1) * 8],
                                         rhs=svcol[:, jc:jc + 1],
                                         start=(jc == 0), stop=(jc == 3))
                # gate math (fused dirs; ops are [128, 32])
                rs = sp.tile([128, 32], BF, tag="rs")
                nc.scalar.activation(rs, przs[:, 0:32], AF.Sigmoid)
                t1 = sp.tile([128, 32], BF, tag="t1")
                nc.vector.tensor_tensor(t1, rs, pns, op=ALU.mult)
                npre = sp.tile([128, 32], BF, tag="np")
                nc.vector.tensor_add(npre.rearrange("p (c j) -> p c j", c=4),
                                     t1.rearrange("p (c j) -> p c j", c=4),
                                     gisT_r[:, 8:12, 8 * s:8 * s + 8])
                nn = sp.tile([128, 32], BF, tag="nn")
                nc.scalar.activation(nn, npre, AF.Tanh)
                zs = sp.tile([128, 32], BF, tag="zs")
                nc.scalar.activation(zs, przs[:, 32:64], AF.Sigmoid)
                omz = sp.tile([128, 32], BF, tag="omz")
                nc.vector.tensor_scalar(out=omz, in0=zs, scalar1=-1.0,
                                        scalar2=1.0, op0=ALU.mult, op1=ALU.add)
                t2s = sp.tile([128, 32], BF, tag="t2s")
                nc.vector.tensor_tensor(t2s, zs, hs, op=ALU.mult)
                a1s = sp.tile([128, 32], BF, tag="a1s")
                nc.vector.tensor_tensor(a1s, omz, nn, op=ALU.mult)
                nc.vector.tensor_add(hs, a1s, t2s)
                # h history (token-major): transpose to base-0 psum, copy out
                ptx_t = ptxp.tile([8, 1024], BF, tag="ptx")    # bank-padded
                ptx = ptx_t[:, 0:512]
                for c in range(4):
                    nc.tensor.transpose(ptx[:, c * 128:(c + 1) * 128],
                                        in_=hs[:, c * 8:(c + 1) * 8],
                                        identity=ident)
                hcp = (s, ptx)   # hstok copy deferred below (Act, post-u)
                # attention u for this step's h
                pus_t = pusp.tile([128, 512], F32, tag="pus")  # bank-padded
                pus = pus_t[:, 0:32]
                for jc in range(4):
                    nc.tensor.matmul(pus[:, jc * 8:(jc + 1) * 8],
                                     lhsT=sbarow[0:1, jc * 128:(jc + 1) * 128],
                                     rhs=ones[:, 0:8], start=(jc == 0), stop=False)
                    for c in range(4):
                        nc.tensor.matmul(
                            pus[:, jc * 8:(jc + 1) * 8],
                            lhsT=sawT[:, c * 512 + jc * 128:c * 512 + (jc + 1) * 128],
                            rhs=hs[:, c * 8:(c + 1) * 8],
                            start=False, stop=(jc == 3 and c == 3))
                us = sp.tile([128, 32], BF, tag="us")
                nc.scalar.activation(us, pus, AF.Tanh)
                spending = (us, s)
                nc.vector.tensor_copy(hstok[:, s * 512:s * 512 + 256],
                                      hcp[1][:, 0:256])
                nc.vector.tensor_copy(hstok[:, s * 512 + 256:s * 512 + 512],
                                      hcp[1][:, 256:512])
            us_, col = spending
            for jc in range(4):
                nc.tensor.matmul(pscs[:, col:col + 1],
                                 lhsT=us_[:, jc * 8:(jc + 1) * 8],
                                 rhs=svcol[:, jc:jc + 1],
                                 start=(jc == 0), stop=(jc == 3))

            # sentence softmax + weighted sum + classifier + log_softmax
            nmx = sp.tile([8, 1], F32, tag="snmx")
            nc.vector.tensor_reduce(nmx, pscs, axis=mybir.AxisListType.X,
                                    op=ALU.max, negate=True)
            ew = sp.tile([8, 16], F32, tag="sew")
            se = sp.tile([8, 1], F32, tag="sse")
            nc.scalar.activation(ew, pscs, AF.Exp, bias=nmx, accum_out=se)
            rse = sp.tile([8, 1], F32, tag="srse")
            nc.vector.reciprocal(rse, se)
            aws = sp.tile([8, 16], F32, tag="saw")
            nc.vector.tensor_scalar_mul(aws, ew, rse)
            pdoc_t = pgssp.tile([128, 512], F32, tag="przs")  # reuse przs bank
            pdoc = pdoc_t[:, 0:32]
            for s in range(16):
                dg = sp.tile([8, 8], BF, tag=f"sdg{s % 8}")
                nc.vector.tensor_scalar_mul(dg, ident[0:8, 0:8], aws[:, s:s + 1])
                for c in range(4):
                    nc.tensor.matmul(pdoc[:, c * 8:(c + 1) * 8],
                                     lhsT=hstok[:, s * 512 + c * 128:s * 512 + (c + 1) * 128],
                                     rhs=dg, start=(s == 0 and c == 0),
                                     stop=(s == 15 and c == 3))
            nc.vector.tensor_copy(doc_sb, pdoc)
            pcl_t = pgssp.tile([128, 512], F32, tag="pns")    # reuse pns bank
            pl = pcl_t[0:10, 0:8]
            plt = pcl_t[0:8, 16:26]
            nc.tensor.matmul(pl, lhsT=fcb[0:1, :], rhs=ones[:, 0:8],
                             start=True, stop=False)
            for c in range(4):
                nc.tensor.matmul(pl, lhsT=fcwT[:, c * NCLS:(c + 1) * NCLS],
                                 rhs=doc_sb[:, c * 8:(c + 1) * 8],
                                 start=False, stop=(c == 3))
            lg = sp.tile([10, 8], F32, tag="lg")
            nc.vector.tensor_copy(lg, pl)
            nc.tensor.transpose(plt, in_=lg, identity=ident32[0:10, 0:10])
            nmx2 = sp.tile([8, 1], F32, tag="nmx2")
            nc.vector.tensor_reduce(nmx2, plt, axis=mybir.AxisListType.X,
                                    op=ALU.max, negate=True)
            e2 = sp.tile([8, NCLS], F32, tag="e2")
            se2 = sp.tile([8, 1], F32, tag="se2")
            nc.scalar.activation(e2, plt, AF.Exp, bias=nmx2, accum_out=se2)
            lse = sp.tile([8, 1], F32, tag="lse")
            nc.scalar.activation(lse, se2, AF.Ln)
            out_sb = sp.tile([8, NCLS], F32, tag="out_sb")
            nc.vector.tensor_scalar(out=out_sb, in0=plt, scalar1=nmx2,
                                    scalar2=lse, op0=ALU.add, op1=ALU.subtract)
            nc.sync.dma_start(out=d["out_d"].ap(), in_=out_sb)


# ---------------------------------------------------------------------------
# host side
# ---------------------------------------------------------------------------

def _prep_inputs(inputs):
    """Build the per-core in_maps (host preprocessing + sharding)."""
    f32 = np.float32
    emb = np.asarray(inputs["emb"], f32)
    w_Wih = np.asarray(inputs["w_Wih"], f32)
    w_Whh = np.asarray(inputs["w_Whh"], f32)
    w_bih = np.asarray(inputs["w_bih"], f32)
    w_bhh = np.asarray(inputs["w_bhh"], f32)
    wa_W = np.asarray(inputs["wa_W"], f32)
    wa_b = np.asarray(inputs["wa_b"], f32)
    wa_v = np.asarray(inputs["wa_v"], f32)
    s_Wih = np.asarray(inputs["s_Wih"], f32)
    s_Whh = np.asarray(inputs["s_Whh"], f32)
    s_bih = np.asarray(inputs["s_bih"], f32)
    s_bhh = np.asarray(inputs["s_bhh"], f32)
    sa_W = np.asarray(inputs["sa_W"], f32)
    sa_b = np.asarray(inputs["sa_b"], f32)
    sa_v = np.asarray(inputs["sa_v"], f32)
    fc_W = np.asarray(inputs["fc_W"], f32)
    fc_b = np.asarray(inputs["fc_b"], f32)
    tokens = np.asarray(inputs["tokens"])

    def b(x):
        return np.ascontiguousarray(x.astype(bf16))

    # folded gather table G [V, 1536], cols [r0|r1|z0|z1|n0|n1]
    g0 = emb @ w_Wih[0].T + w_bih[0]
    g0[:, :512] += w_bhh[0][:512]
    g1 = emb @ w_Wih[1].T + w_bih[1]
    g1[:, :512] += w_bhh[1][:512]
    G = np.concatenate([g0[:, 0:256], g1[:, 0:256],
                        g0[:, 256:512], g1[:, 256:512],
                        g0[:, 512:768], g1[:, 512:768]], 1)

    whhT = np.stack([w_Whh[0].T[:128], w_Whh[0].T[128:],
                     w_Whh[1].T[:128], w_Whh[1].T[128:]])  # [4,128,768]
    brow = np.concatenate([w_bhh[0][512:], w_bhh[1][512:]])[None, :]
    vcol = np.ascontiguousarray(wa_v.reshape(4, 128).T)     # [128, 4]

    # sentence input proj: swihT [512, 1536] cols [r0|r1|z0|z1|n0|n1]
    sg0 = s_Wih[0].T  # [512, 768]
    sg1 = s_Wih[1].T
    swihT = np.concatenate([sg0[:, 0:256], sg1[:, 0:256],
                            sg0[:, 256:512], sg1[:, 256:512],
                            sg0[:, 512:768], sg1[:, 512:768]], 1)
    swT = np.stack([swihT[k * 128:(k + 1) * 128] for k in range(4)])
    sb0 = s_bih[0] + s_bhh[0]
    sb1 = s_bih[1] + s_bhh[1]
    sbirow = np.concatenate([
        sb0[0:256], sb1[0:256], sb0[256:512], sb1[256:512],
        s_bih[0][512:768], s_bih[1][512:768]])[None, :]
    swhhT = np.stack([s_Whh[0].T[:128], s_Whh[0].T[128:],
                      s_Whh[1].T[:128], s_Whh[1].T[128:]])
    sbrow = np.concatenate([s_bhh[0][512:], s_bhh[1][512:]])[None, :]
    svcol = np.ascontiguousarray(sa_v.reshape(4, 128).T)

    shared = {
        "G": b(G), "whhT": b(whhT), "brow": b(brow),
        "waT": b(wa_W.T), "barow": b(wa_b[None, :]), "vcol": b(vcol),
        "swT": b(swT), "sbirow": b(sbirow), "swhhT": b(swhhT),
        "sbrow": b(sbrow), "sawT": b(sa_W.T), "sbarow": b(sa_b[None, :]),
        "svcol": b(svcol), "fcwT": b(fc_W.T), "fcb": b(fc_b[None, :]),
    }
    in_maps = []
    for c in range(NCORES):
        # word-row p = s*8 + doc
        tk = np.ascontiguousarray(
            np.transpose(tokens[c * BC:(c + 1) * BC], (1, 0, 2))
            .reshape(NW, W).astype(np.int32))
        in_maps.append({**shared, "toks": tk})
    return in_maps


_NC_CACHE = {}


def _get_nc():
    if "nc" not in _NC_CACHE:
        _NC_CACHE["nc"] = _build_program()
    return _NC_CACHE["nc"]


def kernel(**inputs) -> np.ndarray:
    nc = _get_nc()
    in_maps = _prep_inputs(inputs)
    res = bass_utils.run_bass_kernel_spmd(nc, in_maps, core_ids=list(range(NCORES)))
    outs = []
    for c in range(NCORES):
        o = np.asarray(res.results[c]["out"], np.float32)
        outs.append(o)
    return np.concatenate(outs, 0)
